# revision 65
# baseline (speedup 1.0000x reference)
"""Trainium2 Bass kernel: 4-layer MLP (784-512-512-512-10) + log_softmax.

Data-parallel over 8 NeuronCores: batch 65536 is split into 8 shards of
8192 rows; the ~1M-param weights are replicated on every core.

v5 schedule: batch-half-major pipeline with 1-bank PSUM groups.
  * Every (m-chunk, batch-half) matmul group accumulates into its own
    2KB PSUM bank (pool of 8); consumers run one half-block (~3us)
    behind producers, so neither the PE nor its LDWEIGHTS ever waits on
    a ReLU drain (the v3 failure mode: 4 simultaneously-opened 2-bank
    groups stalled the in-order PE queue on PSUM WAR ~1.5us/superchunk
    and the micro-gaps re-throttled the clock to 1.2 GHz).
  * Layer 1's K=16 remainder opens each group via row-tiled
    (tile_position=(32m,0)) matmuls, 4 running concurrently in distinct
    PE row-groups, instead of burning 8 full 512-cycle slots.
  * bias+ReLU alternates ScalarE/DVE per (m, half) so both engines stay
    under the PE's pace; exp/sum-of-exp for superchunk sc-1 is issued
    mid-superchunk (never ahead of ReLUs the PE needs); ln+subtract+
    store happen in two bulk epilogues (ScalarE activation-table swaps
    for LN cost 1.3us each, so they must not recur per superchunk).
  * Layer 4 runs one superchunk behind; all DRAM buffers host-packed
    for contiguous-per-partition DMA; output leaves in the flat SBUF
    layout [128, 64, 10] and is un-permuted on host.
v5 head/tail work (the steady state is within ~4% of the fp8-DoubleRow
PE roofline, so the wins are at the edges):
  * The HAM clock gate only reaches 2.4 GHz after a gapless 3.4us busy
    window, and ANY PE gap resets it (costing up to 2 windows) — so the
    head is built to keep the PE gapless from the first possible issue:
    warm-up matmuls sized to bridge the initial DMA wait, initial loads
    interleaved across the sync+scalar DMA queues in first-use order
    (the startup is HBM-bound, so order beats parallelism), and
    superchunk 0 consumed kp-major AND half-interleaved so each k-pair
    is used twice before the next must land.
  * Activation tiles are [128, HB, 2, NB] so per-half reads stay
    contiguous — strided mid-dim slices get dep-coarsened to the whole
    tile, which made layer N+1 wait on BOTH halves' ReLUs.
  * The last superchunk's L4-hb0 runs inside L3's shadow (own PSUM
    tile; a shared bank WAR-stalls hb1's matmuls on hb0's softmax
    reads), and its hb1 m2/m3 ReLUs go to ScalarE so the DVE softmax
    chain can't head-of-line-block the ReLUs the final L4 half waits
    on (the first LDW's wait is coarsened to ALL four h3 ReLUs).

Measured on axon trn2 (fast-clock state; whole-chip clock varies
~16% run-to-run): 172.8us (v1) -> 155.2 (v3) -> 138.8 (v4) -> ~135 (v5).
"""

from contextlib import ExitStack

import ml_dtypes
import numpy as np

import concourse.bass as bass  # noqa: F401  (registers AP machinery)
from concourse import bacc, mybir
from concourse.bass_utils import run_bass_kernel_spmd
from concourse.tile import TileContext

BF16 = mybir.dt.bfloat16
FP32 = mybir.dt.float32
FP8 = mybir.dt.float8e4

N_CORES = 8
B = 65536
D0, H, C = 784, 512, 10
BC = B // N_CORES            # 8192 rows per core
NB = 512                     # matmul moving free dim / PSUM bank width
HB = 2                       # batch halves per superchunk
SNB = NB * HB                # 1024-row superchunk
NCHUNK = BC // SNB           # 8 superchunks
K0F = 6                      # full 128-row contraction chunks in layer 1
K0R = D0 - K0F * 128         # 16 remainder rows
KH = H // 128                # 4 contraction chunks for hidden layers
MG = SNB // 128              # 8 row-groups per superchunk
NRG = BC // 128              # 64 row-groups of 128 rows per core
N_WARM = 24                  # PE warm-up matmuls bridging the initial DMA wait

_CACHED_NC = None


def build_nc():
    nc = bacc.Bacc(
        "TRN2",
        target_bir_lowering=False,
        debug=False,
        enable_asserts=False,
        num_devices=N_CORES,
    )
    xm_d = nc.declare_dram_parameter("xmain", [NCHUNK * 128, K0F * SNB], FP8, isOutput=False)
    xr_d = nc.declare_dram_parameter("xrem", [NCHUNK * 128, SNB], FP8, isOutput=False)
    w1_d = nc.declare_dram_parameter("w1p", [128, K0F * H], FP8, isOutput=False)
    w1r_d = nc.declare_dram_parameter("w1r", [128, 128], FP8, isOutput=False)
    w2_d = nc.declare_dram_parameter("w2p", [128, KH * H], FP8, isOutput=False)
    w3_d = nc.declare_dram_parameter("w3p", [128, KH * H], FP8, isOutput=False)
    w4_d = nc.declare_dram_parameter("w4p", [128, KH * C], BF16, isOutput=False)
    bal_d = nc.declare_dram_parameter("ball", [128, 3 * KH + C], FP32, isOutput=False)
    out_d = nc.declare_dram_parameter("out", [128, NRG, C], FP32, isOutput=True)

    expf = mybir.ActivationFunctionType.Exp
    reluf = mybir.ActivationFunctionType.Relu
    lnf = mybir.ActivationFunctionType.Ln
    add_op = mybir.AluOpType.add
    max_op = mybir.AluOpType.max
    sub_op = mybir.AluOpType.subtract
    mult_op = mybir.AluOpType.mult
    drow = mybir.MatmulPerfMode.DoubleRow

    with TileContext(nc) as tc, ExitStack() as ctx:
        consts = ctx.enter_context(tc.tile_pool(name="consts", bufs=1))
        xpool = ctx.enter_context(tc.tile_pool(name="xp", bufs=3))
        hpool = ctx.enter_context(tc.tile_pool(name="hp", bufs=3))
        spool = ctx.enter_context(tc.tile_pool(name="sp", bufs=2))
        pbig = ctx.enter_context(tc.tile_pool(name="pbig", bufs=8, space="PSUM"))

        # PE warm-up state first: the memset goes on the (otherwise idle) DVE
        # queue so the first warm-up matmul can issue the moment the
        # framework preamble ends — nothing may queue ahead of it. The tile
        # is deliberately small ([128,128], ~120ns memset): warm-ups start
        # ~0.3us earlier than with a 512-wide tile, and the fine-grained
        # matmuls quantize the bridge-to-first-data with less overshoot.
        warm0 = consts.tile([128, 128], FP8, tag="warm0", name="warm0")
        nc.vector.memset(warm0[:], 1.0)
        negone = consts.tile([128, 1], FP32, tag="negone", name="negone")
        nc.vector.memset(negone[:], -1.0)

        # Initial loads: the whole startup is HBM-bandwidth-bound (~340 GB/s
        # across queues), so everything is ordered by first-use time and
        # spread over exactly two queues (sync | scalar). Single k-chunks of
        # superchunk 0's x interleave with the w1 k-pair chunks so that
        # (w1 kp_j + xt kp_j) land just-in-time for the kp-major matmuls;
        # w2-w4 trail at the end, arriving well before L2 of superchunk 0.
        w1r = consts.tile([128, 128], FP8, tag="w1r", name="w1r")
        nc.scalar.dma_start(w1r[:], w1r_d[:])
        xr0 = xpool.tile([128, SNB], FP8, tag="xr", name="xr")
        nc.sync.dma_start(xr0[:], xr_d[0:128, :])
        xt0 = xpool.tile([128, K0F, SNB], FP8, tag="xt", name="xt")
        w1 = consts.tile([128, K0F, H], FP8, tag="w1", name="w1")

        def xt0_chunk(queue, k):
            queue.dma_start(
                xt0[:, k : k + 1, :], xm_d[0:128, k * SNB : (k + 1) * SNB]
            )

        ball = consts.tile([128, 3 * KH + C], FP32, tag="ball", name="ball")
        xt0_chunk(nc.sync, 0)
        nc.scalar.dma_start(w1[:, 0:2, :], w1_d[:, 0 : 2 * H])
        xt0_chunk(nc.sync, 1)
        nc.scalar.dma_start(ball[:], bal_d[:])
        nc.scalar.dma_start(w1[:, 2:4, :], w1_d[:, 2 * H : 4 * H])
        xt0_chunk(nc.sync, 2)
        xt0_chunk(nc.scalar, 3)
        nc.scalar.dma_start(w1[:, 4:6, :], w1_d[:, 4 * H : 6 * H])
        xt0_chunk(nc.sync, 4)
        xt0_chunk(nc.scalar, 5)
        w2 = consts.tile([128, KH, H], FP8, tag="w2", name="w2")
        nc.scalar.dma_start(w2[:], w2_d[:])
        w3 = consts.tile([128, KH, H], FP8, tag="w3", name="w3")
        nc.sync.dma_start(w3[:], w3_d[:])
        w4 = consts.tile([128, KH, C], BF16, tag="w4", name="w4")
        nc.sync.dma_start(w4[:], w4_d[:])

        b4s = ball[:, 3 * KH : 3 * KH + C]

        # PE warm-up: dummy matmuls during the initial DMA wait so the HAM
        # clock gate is at 2.4 GHz when real work arrives; the count is tuned
        # so the warm-ups end right as xr/w1r land (more would delay real
        # work, fewer would let the PE idle and reset the HAM busy window).
        psw = pbig.tile([128, 128], FP32, tag="ps", name="ps_warm")
        for i in range(N_WARM):
            nc.tensor.matmul(
                psw[:], lhsT=warm0[:], rhs=warm0[:],
                start=(i == 0), stop=(i == N_WARM - 1),
            )

        # Persistent softmax state: ln+subtract are deferred to two bulk
        # epilogue calls, so ScalarE never swaps activation tables (RELU/EXP
        # share a set, LN does not) inside the steady-state loop.
        logits_all = consts.tile([128, NRG, C], FP32, tag="logits_all", name="logits_all")
        esum_all = consts.tile([128, NRG], FP32, tag="esum_all", name="esum_all")
        lns_all = consts.tile([128, NRG], FP32, tag="lns_all", name="lns_all")
        obuf = consts.tile([128, NRG, C], FP32, tag="obuf", name="obuf")

        def relu_half(ps, out, bias_ap, on_scalar, split=False):
            if split:
                # halve the ReLU's latency by running its two column halves
                # on both engines in parallel — used only for the very last
                # h3 ReLU, which gates the final layer-4 half.
                h = NB // 2
                nc.scalar.activation(out[:, 0:h], ps[:, 0:h], reluf, bias=bias_ap)
                nc.vector.tensor_scalar(
                    out[:, h:NB], ps[:, h:NB], bias_ap, 0.0, add_op, max_op
                )
                return
            if on_scalar:
                nc.scalar.activation(out, ps[:], reluf, bias=bias_ap)
            else:
                nc.vector.tensor_scalar(out, ps[:], bias_ap, 0.0, add_op, max_op)

        def l4_matmuls_half(h3, ps4h, hb):
            # one batch-half of layer 4 into its own PSUM tile [128, MG/2, C]
            for mm in range(NB // 128):
                ms = slice(mm * 128, (mm + 1) * 128)
                for k in range(KH):
                    nc.tensor.matmul(
                        ps4h[:, mm, :], lhsT=h3[k][:, hb, ms], rhs=w4[:, k, :],
                        start=(k == 0), stop=(k == KH - 1),
                    )

        def l4_matmuls(h3, ps4):
            for hb in range(HB):
                for mm in range(NB // 128):
                    r = hb * (NB // 128) + mm
                    ms = slice(mm * 128, (mm + 1) * 128)
                    for k in range(KH):
                        nc.tensor.matmul(
                            ps4[:, r, :], lhsT=h3[k][:, hb, ms], rhs=w4[:, k, :],
                            start=(k == 0), stop=(k == KH - 1),
                        )

        def l4_softmax_state(rg0, ps_ap, n):
            # logits + exp + sum(exp) for row-groups [rg0, rg0 + n).
            lg = logits_all[:, rg0 : rg0 + n, :]
            nc.vector.tensor_tensor(
                lg, ps_ap, b4s[:, None, :].to_broadcast((128, n, C)), add_op,
            )
            etile = spool.tile([128, n, C], FP32, tag="etile", name="etile")
            nc.scalar.activation(etile[:], lg, expf)
            nc.vector.tensor_reduce(
                esum_all[:, rg0 : rg0 + n], etile[:],
                axis=mybir.AxisListType.X, op=add_op,
            )

        def softmax_epilogue(rg0, rg1, split_store=False):
            # out = logits - ln(sum(exp(logits))) for row-groups [rg0, rg1).
            # ln is computed WITHOUT the Ln activation (whose table set
            # excludes Exp, so each use would cost two 1.28us table swaps):
            # y0 = bits(s)*ln2/2^23 - 126.9427*ln2 - 1 (Mitchell estimate,
            # pre-decremented), then one Newton step ln(s) ~ y0 + s*e^(-y0-1)
            # via the Exp activation that shares the ReLU/Exp table.
            n = rg1 - rg0
            # Private copy first: every fast-ln op then reads DVE-locally
            # written tiles (bitcast views of cross-engine subtile writes
            # are not trusted by the dep tracker).
            esc = spool.tile([128, n], FP32, tag=f"esc_{rg0}", name="esc")
            nc.vector.tensor_copy(esc[:], esum_all[:, rg0:rg1])
            y0 = spool.tile([128, n], FP32, tag=f"y0_{rg0}", name="y0")
            nc.vector.tensor_scalar(
                y0[:], esc[:].bitcast(mybir.dt.int32),
                8.2629582e-8, -88.98996728, mult_op, add_op,
            )
            u = spool.tile([128, n], FP32, tag=f"u_{rg0}", name="u")
            nc.scalar.activation(u[:], y0[:], expf, bias=negone[:, 0:1], scale=-1.0)
            v = spool.tile([128, n], FP32, tag=f"v_{rg0}", name="v")
            nc.vector.tensor_tensor(v[:], esc[:], u[:], mult_op)
            nc.vector.tensor_tensor(lns_all[:, rg0:rg1], y0[:], v[:], add_op)
            # split_store (used only by the very last call, where nothing
            # else overlaps): the first half's store DMA runs while the
            # second half's subtract is still on the DVE.
            spans = (
                [(rg0, (rg0 + rg1) // 2), ((rg0 + rg1) // 2, rg1)]
                if split_store else [(rg0, rg1)]
            )
            for a, b in spans:
                nc.vector.tensor_tensor(
                    obuf[:, a:b, :], logits_all[:, a:b, :],
                    lns_all[:, a:b, None].to_broadcast((128, b - a, C)), sub_op,
                )
                nc.sync.dma_start(out_d[:, a:b, :], obuf[:, a:b, :])

        h3_prev = None
        ps4_prev = None

        def dma_x(sc):
            xr = xpool.tile([128, SNB], FP8, tag="xr", name="xr")
            nc.sync.dma_start(xr[:], xr_d[sc * 128 : (sc + 1) * 128, :])
            xt = xpool.tile([128, K0F, SNB], FP8, tag="xt", name="xt")
            for j in range(K0F // 2):
                nc.sync.dma_start(
                    xt[:, 2 * j : 2 * j + 2, :],
                    xm_d[sc * 128 : (sc + 1) * 128, j * 2 * SNB : (j + 1) * 2 * SNB],
                )
            return xr, xt

        def l1_rem(ps1, xr, hb):
            # K=16 remainder opens all 4 m-groups concurrently (distinct PE
            # row-groups).
            bsl = slice(hb * NB, (hb + 1) * NB)
            for m in range(KH):
                nc.tensor.matmul(
                    ps1[m][:], lhsT=w1r[32 * m : 32 * m + K0R, :],
                    rhs=xr[32 * m : 32 * m + K0R, bsl],
                    start=True, stop=False, perf_mode=None,
                    tile_position=(32 * m, 0),
                )

        def alloc_ps1(hb):
            return [
                pbig.tile([128, NB], FP32, tag="ps", name=f"ps1_{m}_{hb}")
                for m in range(KH)
            ]

        nonlocal_state = {"x_next": (xr0, xt0), "ps1h0_next": None}

        for sc in range(NCHUNK):
            xr, xt = nonlocal_state["x_next"]
            ps1h0_next = nonlocal_state["ps1h0_next"]

            # Layer 1 [784 -> 512], one batch-half at a time. The h0
            # remainder matmuls were issued early (mid-previous-superchunk)
            # so their PSUM WAR deps are long clear and they stay 4-way
            # concurrent; superchunk 0 runs kp-major so it can start on the
            # first-arriving x/w1 k-pair chunk.
            # activation tiles are [128, HB, 2, NB]: the per-half slice
            # [:, hb, :, :] is contiguous, so the dep tracker attributes it
            # to exactly that half's ReLUs (a strided mid-dim slice would be
            # coarsened to the whole tile, making the next layer's matmuls
            # wait on BOTH halves).
            h1p = [
                hpool.tile([128, HB, 2, NB], FP8, tag=f"h1p_{j}", name=f"h1p_{j}")
                for j in range(KH // 2)
            ]
            if sc == 0:
                # Superchunk 0 is paced by the initial HBM loads: matmuls run
                # kp-major AND interleaved across batch halves, so each
                # k-pair is consumed twice before the next one must land.
                # The PE stays gapless on just-in-time data, which also ramps
                # the HAM clock gate at the earliest possible window.
                ps1h = []
                for hb in range(HB):
                    ps1 = alloc_ps1(hb)
                    l1_rem(ps1, xr, hb)
                    ps1h.append(ps1)
                for k in range(0, K0F, 2):
                    for hb in range(HB):
                        bsl = slice(hb * NB, (hb + 1) * NB)
                        for m in range(KH):
                            ms = slice(m * 128, (m + 1) * 128)
                            nc.tensor.matmul(
                                ps1h[hb][m][:], lhsT=w1[:, k : k + 2, ms],
                                rhs=xt[:, k : k + 2, bsl],
                                start=False, stop=(k == K0F - 2), perf_mode=drow,
                            )
                            if k == K0F - 2:
                                # per-m ReLU issue: L2's first matmul needs
                                # all four hb0 ReLUs, so start each the
                                # moment its accumulation closes.
                                relu_half(
                                    ps1h[hb][m], h1p[m // 2][:, hb, m % 2, :],
                                    ball[:, m : m + 1],
                                    on_scalar=((m + hb) % 2 == 0),
                                )
            else:
                for hb in range(HB):
                    bsl = slice(hb * NB, (hb + 1) * NB)
                    if hb == 0 and ps1h0_next is not None:
                        ps1 = ps1h0_next
                    else:
                        ps1 = alloc_ps1(hb)
                        l1_rem(ps1, xr, hb)
                    for m in range(KH):
                        ms = slice(m * 128, (m + 1) * 128)
                        for k in range(0, K0F, 2):
                            nc.tensor.matmul(
                                ps1[m][:], lhsT=w1[:, k : k + 2, ms],
                                rhs=xt[:, k : k + 2, bsl],
                                start=False, stop=(k == K0F - 2), perf_mode=drow,
                            )
                    for m in range(KH):
                        relu_half(
                            ps1[m], h1p[m // 2][:, hb, m % 2, :],
                            ball[:, m : m + 1], on_scalar=((m + hb) % 2 == 0),
                        )

            # Layer 4 of the previous superchunk (its inputs are long ready).
            if h3_prev is not None:
                l4_matmuls(h3_prev, ps4_prev)

            def hidden_layer(
                w, src, dsts, bias_base, out_of_h3,
                after_relu=None, relu_on_scalar=None, relu_split=None,
            ):
                for hb in range(HB):
                    ps = [
                        pbig.tile([128, NB], FP32, tag="ps", name=f"psh_{m}_{hb}")
                        for m in range(KH)
                    ]
                    for m in range(KH):
                        ms = slice(m * 128, (m + 1) * 128)
                        for j in range(KH // 2):
                            nc.tensor.matmul(
                                ps[m][:], lhsT=w[:, 2 * j : 2 * j + 2, ms],
                                rhs=src[j][:, hb, :, :],
                                start=(j == 0), stop=(j == KH // 2 - 1),
                                perf_mode=drow,
                            )
                        out = (
                            dsts[m][:, hb, :] if out_of_h3
                            else dsts[m // 2][:, hb, m % 2, :]
                        )
                        relu_half(
                            ps[m], out, ball[:, bias_base + m : bias_base + m + 1],
                            on_scalar=(
                                ((m + hb) % 2 == 0) if relu_on_scalar is None
                                else relu_on_scalar(hb, m)
                            ),
                            split=(
                                relu_split is not None and relu_split(hb, m)
                            ),
                        )
                        if after_relu is not None:
                            after_relu(hb, m)
                    if hb == 0 and out_of_h3 is False and h3_prev is not None:
                        # exp/sum(exp) of the previous superchunk: issued
                        # mid-superchunk so its ScalarE/DVE ops never queue
                        # ahead of ReLUs the PE is about to wait on.
                        l4_softmax_state((sc - 1) * MG, ps4_prev[:, 0:MG, :], MG)
                        if sc == NCHUNK - 1:
                            # Bulk ln+subtract+store for superchunks 0-6;
                            # the L3 consumers of the ReLUs this delays are
                            # a full half-block behind, so the activation-
                            # table swap hides here.
                            softmax_epilogue(0, (NCHUNK - 1) * MG)
                    if hb == 0 and out_of_h3 and 0 < sc < NCHUNK - 1:
                        # Prefetch next superchunk's x and open its L1-h0
                        # groups now: the PSUM buffers these claim drained
                        # ~4us ago, so the remainder matmuls issue wait-free
                        # and 4-way concurrent. (sc0's prefetch and rem
                        # pre-issue happen earlier — see the sc==0 blocks —
                        # because its compressed, DMA-gated timeline leaves
                        # the mid-L3 points too little headroom, and its
                        # L2->L3 seam has no L4-prev work to hide the
                        # ReLU-wait.)
                        nonlocal_state["x_next"] = dma_x(sc + 1)
                        ps1n = alloc_ps1(0)
                        l1_rem(ps1n, nonlocal_state["x_next"][0], 0)
                        nonlocal_state["ps1h0_next"] = ps1n

            # Layer 2 [512 -> 512]
            if sc == 0:
                nonlocal_state["x_next"] = dma_x(1)
            h2p = [
                hpool.tile([128, HB, 2, NB], FP8, tag=f"h2p_{j}", name=f"h2p_{j}")
                for j in range(KH // 2)
            ]
            hidden_layer(w2, h1p, h2p, KH, out_of_h3=False)

            # Layer 3 [512 -> 512], bf16 out (layer-4 lhsT)
            if sc == 0:
                # sc1's rem pre-issue lands at sc0's L2->L3 seam: with no
                # L4-prev work here, these 4 wait-free row-tiled matmuls
                # (xr landed ~2us ago via the early prefetch) keep the PE
                # busy while L3's first group waits on L2's hb1 ReLUs.
                ps1n = alloc_ps1(0)
                l1_rem(ps1n, nonlocal_state["x_next"][0], 0)
                nonlocal_state["ps1h0_next"] = ps1n
            h3 = [
                hpool.tile([128, HB, NB], BF16, tag=f"h3_{m}", name=f"h3_{m}")
                for m in range(KH)
            ]

            if sc < NCHUNK - 1:
                hidden_layer(w3, h2p, h3, 2 * KH, out_of_h3=True)
                h3_prev = h3
                ps4_prev = pbig.tile([128, MG, C], FP32, tag="ps", name="ps4")
            else:
                # Final superchunk: the hb0 half of L4 issues right after
                # L3's (hb1, m0) matmuls — all hb0 ReLUs have retired by
                # then, so it runs wait-free inside L3's shadow. It gets its
                # OWN PSUM tile (sharing one bank would stall hb1's matmuls
                # on a bank WAR against hb0's softmax reads), allocated at
                # the hook so no later PSUM tenant waits on the very-late
                # softmax reads. The hb1 ReLUs that gate the final L4 half
                # run with m3 alone on ScalarE (m0-m2 on DVE), so the last
                # ReLU starts the instant its matmuls finish; everything
                # else (bias/exp/sum + ln/store for all 8 row-groups) is one
                # short fused chain after it.
                last_state = {}

                def last_sc_hook(hb, m):
                    if hb == 1 and m == 0:
                        ps4a = pbig.tile(
                            [128, MG // 2, C], FP32, tag="ps", name="ps4a"
                        )
                        last_state["ps4a"] = ps4a
                        l4_matmuls_half(h3, ps4a, 0)

                # hb1's m2/m3 ReLUs go to ScalarE: the final L4 half's first
                # LDWEIGHTS gets a wait coarsened to ALL four h3 ReLUs, and
                # the DVE queue behind it also carries the softmax chain —
                # on DVE the scheduler head-of-line-blocks the late ReLUs
                # behind softmax ops that wait on layer-4 PSUM.
                hidden_layer(
                    w3, h2p, h3, 2 * KH, out_of_h3=True,
                    after_relu=last_sc_hook,
                    relu_on_scalar=lambda hb, m: (m >= 2) if hb == 1
                    else ((m + hb) % 2 == 0),
                    relu_split=lambda hb, m: hb == 1 and m == 3,
                )
                ps4a = last_state["ps4a"]
                rgb = sc * MG
                # hb0's exp/sum is emitted BEFORE the hb1 matmuls (its input
                # PSUM is long done) so it runs in their shadow; after the
                # last matmul only hb1's short exp/sum and one ln/store
                # chain remain.
                l4_softmax_state(rgb, ps4a[:, :, :], MG // 2)
                ps4b = pbig.tile([128, MG // 2, C], FP32, tag="ps", name="ps4b")
                l4_matmuls_half(h3, ps4b, 1)
                l4_softmax_state(rgb + MG // 2, ps4b[:, :, :], MG // 2)
                softmax_epilogue(rgb, NRG)

    nc.compile()
    return nc


def _get_nc():
    global _CACHED_NC
    if _CACHED_NC is None:
        _CACHED_NC = build_nc()
    return _CACHED_NC


def make_in_maps(x, W1, b1, W2, b2, W3, b3, W4, b4):
    bf16 = ml_dtypes.bfloat16
    fp8 = ml_dtypes.float8_e4m3
    f32 = np.float32
    W1, W2, W3, W4 = (np.asarray(w, dtype=f32) for w in (W1, W2, W3, W4))

    # w1p[p, k*512+m] = W1[m, k*128+p]
    w1p = np.ascontiguousarray(
        W1[:, : K0F * 128].reshape(H, K0F, 128).transpose(2, 1, 0)
    ).reshape(128, K0F * H).astype(fp8)
    # w1r[32i+j, c] = W1[128i+c, 768+j]  (row-tiled remainder weights)
    w1r = np.zeros((128, 128), dtype=fp8)
    wr = W1[:, K0F * 128 :].astype(fp8)  # [512, 16]
    for i in range(KH):
        w1r[32 * i : 32 * i + K0R, :] = wr[128 * i : 128 * (i + 1), :].T
    # w2p[p, o*512+m] = W2[m, o*128+p]
    def packw(W):
        return np.ascontiguousarray(
            W.T.reshape(KH, 128, H).transpose(1, 0, 2)
        ).reshape(128, KH * H).astype(fp8)
    w2p, w3p = packw(W2), packw(W3)
    w4p = np.ascontiguousarray(
        W4.T.reshape(KH, 128, C).transpose(1, 0, 2)
    ).reshape(128, KH * C).astype(bf16)
    ball = np.concatenate(
        [
            np.asarray(b1, f32).reshape(KH, 128).T,
            np.asarray(b2, f32).reshape(KH, 128).T,
            np.asarray(b3, f32).reshape(KH, 128).T,
            np.tile(np.asarray(b4, f32)[None, :], (128, 1)),
        ],
        axis=1,
    )
    common = {
        "w1p": w1p, "w1r": w1r, "w2p": w2p, "w3p": w3p, "w4p": w4p,
        "ball": np.ascontiguousarray(ball),
    }

    xq = np.asarray(x).astype(fp8)
    in_maps = []
    for ci in range(N_CORES):
        xs = xq[ci * BC : (ci + 1) * BC]  # [8192, 784]
        # xmain[sc*128+p, k*1024+b] = xs[sc*1024+b, k*128+p]
        xmain = np.ascontiguousarray(
            xs[:, : K0F * 128].reshape(NCHUNK, SNB, K0F, 128).transpose(0, 3, 2, 1)
        ).reshape(NCHUNK * 128, K0F * SNB)
        # xrem[sc*128+32i+j, b] = xs[sc*1024+b, 768+j], replicated over i
        xrp = xs[:, K0F * 128 :].reshape(NCHUNK, SNB, K0R).transpose(0, 2, 1)
        xrem = np.zeros((NCHUNK, 128, SNB), dtype=fp8)
        for i in range(KH):
            xrem[:, 32 * i : 32 * i + K0R, :] = xrp
        in_maps.append(
            {"xmain": xmain, "xrem": xrem.reshape(NCHUNK * 128, SNB), **common}
        )
    return in_maps


def assemble_output(res):
    # out dram is the flat SBUF layout [128, 64, 10]; row rg*128+p of the
    # core's shard lives at out[p, rg, :].
    parts = []
    for i in range(N_CORES):
        o = np.asarray(res.results[i]["out"], dtype=np.float32)
        parts.append(o.transpose(1, 0, 2).reshape(BC, C))
    return np.concatenate(parts, axis=0)


def kernel(x, W1, b1, W2, b2, W3, b3, W4, b4):
    in_maps = make_in_maps(x, W1, b1, W2, b2, W3, b3, W4, b4)
    nc = _get_nc()
    res = run_bass_kernel_spmd(nc, in_maps, list(range(N_CORES)))
    return assemble_output(res)



# revision 67
# speedup vs baseline: 1.1702x; 1.1702x over previous
"""Trainium2 Bass kernel: 4-layer MLP (784-512-512-512-10) + log_softmax.

Data-parallel over 8 NeuronCores: batch 65536 is split into 8 shards of
8192 rows; the ~1M-param weights are replicated on every core.

v5 schedule: batch-half-major pipeline with 1-bank PSUM groups.
  * Every (m-chunk, batch-half) matmul group accumulates into its own
    2KB PSUM bank (pool of 8); consumers run one half-block (~3us)
    behind producers, so neither the PE nor its LDWEIGHTS ever waits on
    a ReLU drain (the v3 failure mode: 4 simultaneously-opened 2-bank
    groups stalled the in-order PE queue on PSUM WAR ~1.5us/superchunk
    and the micro-gaps re-throttled the clock to 1.2 GHz).
  * Layer 1's K=16 remainder opens each group via row-tiled
    (tile_position=(32m,0)) matmuls, 4 running concurrently in distinct
    PE row-groups, instead of burning 8 full 512-cycle slots.
  * bias+ReLU alternates ScalarE/DVE per (m, half) so both engines stay
    under the PE's pace; exp/sum-of-exp for superchunk sc-1 is issued
    mid-superchunk (never ahead of ReLUs the PE needs); ln+subtract+
    store happen in two bulk epilogues (ScalarE activation-table swaps
    for LN cost 1.3us each, so they must not recur per superchunk).
  * Layer 4 runs one superchunk behind; all DRAM buffers host-packed
    for contiguous-per-partition DMA; output leaves in the flat SBUF
    layout [128, 64, 10] and is un-permuted on host.
v5 head/tail work (the steady state is within ~4% of the fp8-DoubleRow
PE roofline, so the wins are at the edges):
  * The HAM clock gate only reaches 2.4 GHz after a gapless 3.4us busy
    window, and ANY PE gap resets it (costing up to 2 windows) — so the
    head is built to keep the PE gapless from the first possible issue:
    warm-up matmuls sized to bridge the initial DMA wait, initial loads
    interleaved across the sync+scalar DMA queues in first-use order
    (the startup is HBM-bound, so order beats parallelism), and
    superchunk 0 consumed kp-major AND half-interleaved so each k-pair
    is used twice before the next must land.
  * Activation tiles are [128, HB, 2, NB] so per-half reads stay
    contiguous — strided mid-dim slices get dep-coarsened to the whole
    tile, which made layer N+1 wait on BOTH halves' ReLUs.
  * The last superchunk's L4-hb0 runs inside L3's shadow (own PSUM
    tile; a shared bank WAR-stalls hb1's matmuls on hb0's softmax
    reads), and its hb1 m2/m3 ReLUs go to ScalarE so the DVE softmax
    chain can't head-of-line-block the ReLUs the final L4 half waits
    on (the first LDW's wait is coarsened to ALL four h3 ReLUs).

Measured on axon trn2 (fast-clock state; whole-chip clock varies
~16% run-to-run): 172.8us (v1) -> 155.2 (v3) -> 138.8 (v4) -> ~135 (v5).
"""

from contextlib import ExitStack

import ml_dtypes
import numpy as np

import concourse.bass as bass  # noqa: F401  (registers AP machinery)
from concourse import bacc, mybir
from concourse.bass_utils import run_bass_kernel_spmd
from concourse.tile import TileContext

BF16 = mybir.dt.bfloat16
FP32 = mybir.dt.float32
FP8 = mybir.dt.float8e4

N_CORES = 8
B = 65536
D0, H, C = 784, 512, 10
BC = B // N_CORES            # 8192 rows per core
NB = 512                     # matmul moving free dim / PSUM bank width
HB = 2                       # batch halves per superchunk
SNB = NB * HB                # 1024-row superchunk
NCHUNK = BC // SNB           # 8 superchunks
K0F = 6                      # full 128-row contraction chunks in layer 1
K0R = D0 - K0F * 128         # 16 remainder rows
KH = H // 128                # 4 contraction chunks for hidden layers
MG = SNB // 128              # 8 row-groups per superchunk
NRG = BC // 128              # 64 row-groups of 128 rows per core
N_WARM = 24                  # PE warm-up matmuls bridging the initial DMA wait

_CACHED_NC = None


def build_nc():
    nc = bacc.Bacc(
        "TRN2",
        target_bir_lowering=False,
        debug=False,
        enable_asserts=False,
        num_devices=N_CORES,
    )
    xm_d = nc.declare_dram_parameter("xmain", [NCHUNK * 128, K0F * SNB], FP8, isOutput=False)
    xr_d = nc.declare_dram_parameter("xrem", [NCHUNK * 128, SNB], FP8, isOutput=False)
    w1_d = nc.declare_dram_parameter("w1p", [128, K0F * H], FP8, isOutput=False)
    w1r_d = nc.declare_dram_parameter("w1r", [128, 128], FP8, isOutput=False)
    w2_d = nc.declare_dram_parameter("w2p", [128, KH * H], FP8, isOutput=False)
    w3_d = nc.declare_dram_parameter("w3p", [128, KH * H], FP8, isOutput=False)
    w4_d = nc.declare_dram_parameter("w4p", [128, KH * C], BF16, isOutput=False)
    bal_d = nc.declare_dram_parameter("ball", [128, 3 * KH + C], FP32, isOutput=False)
    out_d = nc.declare_dram_parameter("out", [128, NRG, C], FP32, isOutput=True)

    expf = mybir.ActivationFunctionType.Exp
    reluf = mybir.ActivationFunctionType.Relu
    lnf = mybir.ActivationFunctionType.Ln
    add_op = mybir.AluOpType.add
    max_op = mybir.AluOpType.max
    sub_op = mybir.AluOpType.subtract
    mult_op = mybir.AluOpType.mult
    drow = mybir.MatmulPerfMode.DoubleRow

    with TileContext(nc) as tc, ExitStack() as ctx:
        consts = ctx.enter_context(tc.tile_pool(name="consts", bufs=1))
        xpool = ctx.enter_context(tc.tile_pool(name="xp", bufs=3))
        hpool = ctx.enter_context(tc.tile_pool(name="hp", bufs=3))
        spool = ctx.enter_context(tc.tile_pool(name="sp", bufs=2))
        pbig = ctx.enter_context(tc.tile_pool(name="pbig", bufs=8, space="PSUM"))

        # PE warm-up state first: the memset goes on the (otherwise idle) DVE
        # queue so the first warm-up matmul can issue the moment the
        # framework preamble ends — nothing may queue ahead of it. The tile
        # is deliberately small ([128,128], ~120ns memset): warm-ups start
        # ~0.3us earlier than with a 512-wide tile, and the fine-grained
        # matmuls quantize the bridge-to-first-data with less overshoot.
        warm0 = consts.tile([128, 128], FP8, tag="warm0", name="warm0")
        nc.vector.memset(warm0[:], 1.0)
        negone = consts.tile([128, 1], FP32, tag="negone", name="negone")
        nc.vector.memset(negone[:], -1.0)

        # Initial loads: the whole startup is HBM-bandwidth-bound (~340 GB/s
        # across queues), so everything is ordered by first-use time and
        # spread over exactly two queues (sync | scalar). Single k-chunks of
        # superchunk 0's x interleave with the w1 k-pair chunks so that
        # (w1 kp_j + xt kp_j) land just-in-time for the kp-major matmuls;
        # w2-w4 trail at the end, arriving well before L2 of superchunk 0.
        w1r = consts.tile([128, 128], FP8, tag="w1r", name="w1r")
        nc.scalar.dma_start(w1r[:], w1r_d[:])
        xr0 = xpool.tile([128, SNB], FP8, tag="xr", name="xr")
        nc.sync.dma_start(xr0[:], xr_d[0:128, :])
        xt0 = xpool.tile([128, K0F, SNB], FP8, tag="xt", name="xt")
        w1 = consts.tile([128, K0F, H], FP8, tag="w1", name="w1")

        def xt0_chunk(queue, k):
            queue.dma_start(
                xt0[:, k : k + 1, :], xm_d[0:128, k * SNB : (k + 1) * SNB]
            )

        ball = consts.tile([128, 3 * KH + C], FP32, tag="ball", name="ball")
        xt0_chunk(nc.sync, 0)
        nc.scalar.dma_start(w1[:, 0:2, :], w1_d[:, 0 : 2 * H])
        xt0_chunk(nc.sync, 1)
        nc.scalar.dma_start(ball[:], bal_d[:])
        nc.scalar.dma_start(w1[:, 2:4, :], w1_d[:, 2 * H : 4 * H])
        xt0_chunk(nc.sync, 2)
        xt0_chunk(nc.scalar, 3)
        nc.scalar.dma_start(w1[:, 4:6, :], w1_d[:, 4 * H : 6 * H])
        xt0_chunk(nc.sync, 4)
        xt0_chunk(nc.scalar, 5)
        w2 = consts.tile([128, KH, H], FP8, tag="w2", name="w2")
        nc.scalar.dma_start(w2[:], w2_d[:])
        w3 = consts.tile([128, KH, H], FP8, tag="w3", name="w3")
        nc.sync.dma_start(w3[:], w3_d[:])
        w4 = consts.tile([128, KH, C], BF16, tag="w4", name="w4")
        nc.sync.dma_start(w4[:], w4_d[:])

        b4s = ball[:, 3 * KH : 3 * KH + C]

        # PE warm-up: dummy matmuls during the initial DMA wait so the HAM
        # clock gate is at 2.4 GHz when real work arrives; the count is tuned
        # so the warm-ups end right as xr/w1r land (more would delay real
        # work, fewer would let the PE idle and reset the HAM busy window).
        psw = pbig.tile([128, 128], FP32, tag="ps", name="ps_warm")
        for i in range(N_WARM):
            nc.tensor.matmul(
                psw[:], lhsT=warm0[:], rhs=warm0[:],
                start=(i == 0), stop=(i == N_WARM - 1),
            )

        # Persistent softmax state: ln+subtract are deferred to two bulk
        # epilogue calls, so ScalarE never swaps activation tables (RELU/EXP
        # share a set, LN does not) inside the steady-state loop.
        logits_all = consts.tile([128, NRG, C], FP32, tag="logits_all", name="logits_all")
        esum_all = consts.tile([128, NRG], FP32, tag="esum_all", name="esum_all")
        lns_all = consts.tile([128, NRG], FP32, tag="lns_all", name="lns_all")
        obuf = consts.tile([128, NRG, C], FP32, tag="obuf", name="obuf")

        def relu_half(ps, out, bias_ap, on_scalar, split=False):
            if split:
                # halve the ReLU's latency by running its two column halves
                # on both engines in parallel — used only for the very last
                # h3 ReLU, which gates the final layer-4 half.
                h = NB // 2
                nc.scalar.activation(out[:, 0:h], ps[:, 0:h], reluf, bias=bias_ap)
                nc.vector.tensor_scalar(
                    out[:, h:NB], ps[:, h:NB], bias_ap, 0.0, add_op, max_op
                )
                return
            if on_scalar:
                nc.scalar.activation(out, ps[:], reluf, bias=bias_ap)
            else:
                nc.vector.tensor_scalar(out, ps[:], bias_ap, 0.0, add_op, max_op)

        def l4_matmuls_half(h3, ps4h, hb):
            # one batch-half of layer 4 into its own PSUM tile [128, MG/2, C]
            for mm in range(NB // 128):
                ms = slice(mm * 128, (mm + 1) * 128)
                for k in range(KH):
                    nc.tensor.matmul(
                        ps4h[:, mm, :], lhsT=h3[k][:, hb, ms], rhs=w4[:, k, :],
                        start=(k == 0), stop=(k == KH - 1),
                    )

        def l4_matmuls(h3, ps4):
            for hb in range(HB):
                for mm in range(NB // 128):
                    r = hb * (NB // 128) + mm
                    ms = slice(mm * 128, (mm + 1) * 128)
                    for k in range(KH):
                        nc.tensor.matmul(
                            ps4[:, r, :], lhsT=h3[k][:, hb, ms], rhs=w4[:, k, :],
                            start=(k == 0), stop=(k == KH - 1),
                        )

        def l4_softmax_state(rg0, ps_ap, n):
            # logits + exp + sum(exp) for row-groups [rg0, rg0 + n).
            lg = logits_all[:, rg0 : rg0 + n, :]
            nc.vector.tensor_tensor(
                lg, ps_ap, b4s[:, None, :].to_broadcast((128, n, C)), add_op,
            )
            etile = spool.tile([128, n, C], FP32, tag="etile", name="etile")
            nc.scalar.activation(etile[:], lg, expf)
            nc.vector.tensor_reduce(
                esum_all[:, rg0 : rg0 + n], etile[:],
                axis=mybir.AxisListType.X, op=add_op,
            )

        def softmax_epilogue(rg0, rg1, split_store=False):
            # out = logits - ln(sum(exp(logits))) for row-groups [rg0, rg1).
            # ln is computed WITHOUT the Ln activation (whose table set
            # excludes Exp, so each use would cost two 1.28us table swaps):
            # y0 = bits(s)*ln2/2^23 - 126.9427*ln2 - 1 (Mitchell estimate,
            # pre-decremented), then one Newton step ln(s) ~ y0 + s*e^(-y0-1)
            # via the Exp activation that shares the ReLU/Exp table.
            n = rg1 - rg0
            # Private copy first: every fast-ln op then reads DVE-locally
            # written tiles (bitcast views of cross-engine subtile writes
            # are not trusted by the dep tracker).
            esc = spool.tile([128, n], FP32, tag=f"esc_{rg0}", name="esc")
            nc.vector.tensor_copy(esc[:], esum_all[:, rg0:rg1])
            y0 = spool.tile([128, n], FP32, tag=f"y0_{rg0}", name="y0")
            nc.vector.tensor_scalar(
                y0[:], esc[:].bitcast(mybir.dt.int32),
                8.2629582e-8, -88.98996728, mult_op, add_op,
            )
            u = spool.tile([128, n], FP32, tag=f"u_{rg0}", name="u")
            nc.scalar.activation(u[:], y0[:], expf, bias=negone[:, 0:1], scale=-1.0)
            v = spool.tile([128, n], FP32, tag=f"v_{rg0}", name="v")
            nc.vector.tensor_tensor(v[:], esc[:], u[:], mult_op)
            nc.vector.tensor_tensor(lns_all[:, rg0:rg1], y0[:], v[:], add_op)
            # split_store (used only by the very last call, where nothing
            # else overlaps): the first half's store DMA runs while the
            # second half's subtract is still on the DVE.
            spans = (
                [(rg0, (rg0 + rg1) // 2), ((rg0 + rg1) // 2, rg1)]
                if split_store else [(rg0, rg1)]
            )
            for a, b in spans:
                nc.vector.tensor_tensor(
                    obuf[:, a:b, :], logits_all[:, a:b, :],
                    lns_all[:, a:b, None].to_broadcast((128, b - a, C)), sub_op,
                )
                nc.sync.dma_start(out_d[:, a:b, :], obuf[:, a:b, :])

        h3_prev = None
        ps4_prev = None

        def dma_x(sc):
            xr = xpool.tile([128, SNB], FP8, tag="xr", name="xr")
            nc.sync.dma_start(xr[:], xr_d[sc * 128 : (sc + 1) * 128, :])
            xt = xpool.tile([128, K0F, SNB], FP8, tag="xt", name="xt")
            for j in range(K0F // 2):
                nc.sync.dma_start(
                    xt[:, 2 * j : 2 * j + 2, :],
                    xm_d[sc * 128 : (sc + 1) * 128, j * 2 * SNB : (j + 1) * 2 * SNB],
                )
            return xr, xt

        def l1_rem(ps1, xr, hb):
            # K=16 remainder opens all 4 m-groups concurrently (distinct PE
            # row-groups).
            bsl = slice(hb * NB, (hb + 1) * NB)
            for m in range(KH):
                nc.tensor.matmul(
                    ps1[m][:], lhsT=w1r[32 * m : 32 * m + K0R, :],
                    rhs=xr[32 * m : 32 * m + K0R, bsl],
                    start=True, stop=False, perf_mode=None,
                    tile_position=(32 * m, 0),
                )

        def alloc_ps1(hb):
            return [
                pbig.tile([128, NB], FP32, tag="ps", name=f"ps1_{m}_{hb}")
                for m in range(KH)
            ]

        nonlocal_state = {"x_next": (xr0, xt0), "ps1h0_next": None}

        for sc in range(NCHUNK):
            xr, xt = nonlocal_state["x_next"]
            ps1h0_next = nonlocal_state["ps1h0_next"]

            # Layer 1 [784 -> 512], one batch-half at a time. The h0
            # remainder matmuls were issued early (mid-previous-superchunk)
            # so their PSUM WAR deps are long clear and they stay 4-way
            # concurrent; superchunk 0 runs kp-major so it can start on the
            # first-arriving x/w1 k-pair chunk.
            # activation tiles are [128, HB, 2, NB]: the per-half slice
            # [:, hb, :, :] is contiguous, so the dep tracker attributes it
            # to exactly that half's ReLUs (a strided mid-dim slice would be
            # coarsened to the whole tile, making the next layer's matmuls
            # wait on BOTH halves).
            h1p = [
                hpool.tile([128, HB, 2, NB], FP8, tag=f"h1p_{j}", name=f"h1p_{j}")
                for j in range(KH // 2)
            ]
            if sc == 0:
                # Superchunk 0 is paced by the initial HBM loads: matmuls run
                # kp-major AND interleaved across batch halves, so each
                # k-pair is consumed twice before the next one must land.
                # The PE stays gapless on just-in-time data, which also ramps
                # the HAM clock gate at the earliest possible window.
                ps1h = []
                for hb in range(HB):
                    ps1 = alloc_ps1(hb)
                    l1_rem(ps1, xr, hb)
                    ps1h.append(ps1)
                for k in range(0, K0F, 2):
                    for hb in range(HB):
                        bsl = slice(hb * NB, (hb + 1) * NB)
                        for m in range(KH):
                            ms = slice(m * 128, (m + 1) * 128)
                            nc.tensor.matmul(
                                ps1h[hb][m][:], lhsT=w1[:, k : k + 2, ms],
                                rhs=xt[:, k : k + 2, bsl],
                                start=False, stop=(k == K0F - 2), perf_mode=drow,
                            )
                            if k == K0F - 2:
                                # per-m ReLU issue: L2's first matmul needs
                                # all four hb0 ReLUs, so start each the
                                # moment its accumulation closes.
                                relu_half(
                                    ps1h[hb][m], h1p[m // 2][:, hb, m % 2, :],
                                    ball[:, m : m + 1],
                                    on_scalar=((m + hb) % 2 == 0),
                                )
            else:
                for hb in range(HB):
                    bsl = slice(hb * NB, (hb + 1) * NB)
                    if hb == 0 and ps1h0_next is not None:
                        ps1 = ps1h0_next
                    else:
                        ps1 = alloc_ps1(hb)
                        l1_rem(ps1, xr, hb)
                    for m in range(KH):
                        ms = slice(m * 128, (m + 1) * 128)
                        for k in range(0, K0F, 2):
                            nc.tensor.matmul(
                                ps1[m][:], lhsT=w1[:, k : k + 2, ms],
                                rhs=xt[:, k : k + 2, bsl],
                                start=False, stop=(k == K0F - 2), perf_mode=drow,
                            )
                    for m in range(KH):
                        relu_half(
                            ps1[m], h1p[m // 2][:, hb, m % 2, :],
                            ball[:, m : m + 1], on_scalar=((m + hb) % 2 == 0),
                        )

            # Layer 4 of the previous superchunk (its inputs are long ready).
            if h3_prev is not None:
                l4_matmuls(h3_prev, ps4_prev)

            def hidden_layer(
                w, src, dsts, bias_base, out_of_h3,
                after_relu=None, relu_on_scalar=None, relu_split=None,
            ):
                for hb in range(HB):
                    ps = [
                        pbig.tile([128, NB], FP32, tag="ps", name=f"psh_{m}_{hb}")
                        for m in range(KH)
                    ]
                    for m in range(KH):
                        ms = slice(m * 128, (m + 1) * 128)
                        for j in range(KH // 2):
                            nc.tensor.matmul(
                                ps[m][:], lhsT=w[:, 2 * j : 2 * j + 2, ms],
                                rhs=src[j][:, hb, :, :],
                                start=(j == 0), stop=(j == KH // 2 - 1),
                                perf_mode=drow,
                            )
                        out = (
                            dsts[m][:, hb, :] if out_of_h3
                            else dsts[m // 2][:, hb, m % 2, :]
                        )
                        relu_half(
                            ps[m], out, ball[:, bias_base + m : bias_base + m + 1],
                            on_scalar=(
                                ((m + hb) % 2 == 0) if relu_on_scalar is None
                                else relu_on_scalar(hb, m)
                            ),
                            split=(
                                relu_split is not None and relu_split(hb, m)
                            ),
                        )
                        if after_relu is not None:
                            after_relu(hb, m)
                    if hb == 0 and out_of_h3 is False and h3_prev is not None:
                        # exp/sum(exp) of the previous superchunk: issued
                        # mid-superchunk so its ScalarE/DVE ops never queue
                        # ahead of ReLUs the PE is about to wait on.
                        l4_softmax_state((sc - 1) * MG, ps4_prev[:, 0:MG, :], MG)
                        if sc == NCHUNK - 1:
                            # Bulk ln+subtract+store for superchunks 0-6;
                            # the L3 consumers of the ReLUs this delays are
                            # a full half-block behind, so the activation-
                            # table swap hides here.
                            softmax_epilogue(0, (NCHUNK - 1) * MG)
                    if hb == 0 and out_of_h3 and 0 < sc < NCHUNK - 1:
                        # Prefetch next superchunk's x and open its L1-h0
                        # groups now: the PSUM buffers these claim drained
                        # ~4us ago, so the remainder matmuls issue wait-free
                        # and 4-way concurrent. (For sc0 the prefetch itself
                        # was already issued before L2 — its compressed,
                        # DMA-gated timeline leaves the mid-L3 issue point
                        # too little headroom for the 918KB transfer.)
                        if sc > 0:
                            nonlocal_state["x_next"] = dma_x(sc + 1)
                        ps1n = alloc_ps1(0)
                        l1_rem(ps1n, nonlocal_state["x_next"][0], 0)
                        nonlocal_state["ps1h0_next"] = ps1n

            # Layer 2 [512 -> 512]
            if sc == 0:
                nonlocal_state["x_next"] = dma_x(1)
            h2p = [
                hpool.tile([128, HB, 2, NB], FP8, tag=f"h2p_{j}", name=f"h2p_{j}")
                for j in range(KH // 2)
            ]
            hidden_layer(w2, h1p, h2p, KH, out_of_h3=False)

            # Layer 3 [512 -> 512], bf16 out (layer-4 lhsT)
            if sc == 0:
                # sc1's rem pre-issue lands at sc0's L2->L3 seam: with no
                # L4-prev work here, these 4 wait-free row-tiled matmuls
                # (xr landed ~2us ago via the early prefetch) keep the PE
                # busy while L3's first group waits on L2's hb1 ReLUs.
                ps1n = alloc_ps1(0)
                l1_rem(ps1n, nonlocal_state["x_next"][0], 0)
                nonlocal_state["ps1h0_next"] = ps1n
            h3 = [
                hpool.tile([128, HB, NB], BF16, tag=f"h3_{m}", name=f"h3_{m}")
                for m in range(KH)
            ]

            if sc < NCHUNK - 1:
                hidden_layer(w3, h2p, h3, 2 * KH, out_of_h3=True)
                h3_prev = h3
                ps4_prev = pbig.tile([128, MG, C], FP32, tag="ps", name="ps4")
            else:
                # Final superchunk: the hb0 half of L4 issues right after
                # L3's (hb1, m0) matmuls — all hb0 ReLUs have retired by
                # then, so it runs wait-free inside L3's shadow. It gets its
                # OWN PSUM tile (sharing one bank would stall hb1's matmuls
                # on a bank WAR against hb0's softmax reads), allocated at
                # the hook so no later PSUM tenant waits on the very-late
                # softmax reads. The hb1 ReLUs that gate the final L4 half
                # run with m3 alone on ScalarE (m0-m2 on DVE), so the last
                # ReLU starts the instant its matmuls finish; everything
                # else (bias/exp/sum + ln/store for all 8 row-groups) is one
                # short fused chain after it.
                last_state = {}

                def last_sc_hook(hb, m):
                    if hb == 1 and m == 0:
                        ps4a = pbig.tile(
                            [128, MG // 2, C], FP32, tag="ps", name="ps4a"
                        )
                        last_state["ps4a"] = ps4a
                        l4_matmuls_half(h3, ps4a, 0)

                # hb1's m2/m3 ReLUs go to ScalarE: the final L4 half's first
                # LDWEIGHTS gets a wait coarsened to ALL four h3 ReLUs, and
                # the DVE queue behind it also carries the softmax chain —
                # on DVE the scheduler head-of-line-blocks the late ReLUs
                # behind softmax ops that wait on layer-4 PSUM.
                hidden_layer(
                    w3, h2p, h3, 2 * KH, out_of_h3=True,
                    after_relu=last_sc_hook,
                    relu_on_scalar=lambda hb, m: (m >= 2) if hb == 1
                    else ((m + hb) % 2 == 0),
                    relu_split=lambda hb, m: hb == 1 and m == 3,
                )
                ps4a = last_state["ps4a"]
                rgb = sc * MG
                # hb0's exp/sum is emitted BEFORE the hb1 matmuls (its input
                # PSUM is long done) so it runs in their shadow; after the
                # last matmul only hb1's short exp/sum and one ln/store
                # chain remain.
                l4_softmax_state(rgb, ps4a[:, :, :], MG // 2)
                ps4b = pbig.tile([128, MG // 2, C], FP32, tag="ps", name="ps4b")
                l4_matmuls_half(h3, ps4b, 1)
                l4_softmax_state(rgb + MG // 2, ps4b[:, :, :], MG // 2)
                softmax_epilogue(rgb, NRG)

    nc.compile()
    return nc


def _get_nc():
    global _CACHED_NC
    if _CACHED_NC is None:
        _CACHED_NC = build_nc()
    return _CACHED_NC


def make_in_maps(x, W1, b1, W2, b2, W3, b3, W4, b4):
    bf16 = ml_dtypes.bfloat16
    fp8 = ml_dtypes.float8_e4m3
    f32 = np.float32
    W1, W2, W3, W4 = (np.asarray(w, dtype=f32) for w in (W1, W2, W3, W4))

    # w1p[p, k*512+m] = W1[m, k*128+p]
    w1p = np.ascontiguousarray(
        W1[:, : K0F * 128].reshape(H, K0F, 128).transpose(2, 1, 0)
    ).reshape(128, K0F * H).astype(fp8)
    # w1r[32i+j, c] = W1[128i+c, 768+j]  (row-tiled remainder weights)
    w1r = np.zeros((128, 128), dtype=fp8)
    wr = W1[:, K0F * 128 :].astype(fp8)  # [512, 16]
    for i in range(KH):
        w1r[32 * i : 32 * i + K0R, :] = wr[128 * i : 128 * (i + 1), :].T
    # w2p[p, o*512+m] = W2[m, o*128+p]
    def packw(W):
        return np.ascontiguousarray(
            W.T.reshape(KH, 128, H).transpose(1, 0, 2)
        ).reshape(128, KH * H).astype(fp8)
    w2p, w3p = packw(W2), packw(W3)
    w4p = np.ascontiguousarray(
        W4.T.reshape(KH, 128, C).transpose(1, 0, 2)
    ).reshape(128, KH * C).astype(bf16)
    ball = np.concatenate(
        [
            np.asarray(b1, f32).reshape(KH, 128).T,
            np.asarray(b2, f32).reshape(KH, 128).T,
            np.asarray(b3, f32).reshape(KH, 128).T,
            np.tile(np.asarray(b4, f32)[None, :], (128, 1)),
        ],
        axis=1,
    )
    common = {
        "w1p": w1p, "w1r": w1r, "w2p": w2p, "w3p": w3p, "w4p": w4p,
        "ball": np.ascontiguousarray(ball),
    }

    xq = np.asarray(x).astype(fp8)
    in_maps = []
    for ci in range(N_CORES):
        xs = xq[ci * BC : (ci + 1) * BC]  # [8192, 784]
        # xmain[sc*128+p, k*1024+b] = xs[sc*1024+b, k*128+p]
        xmain = np.ascontiguousarray(
            xs[:, : K0F * 128].reshape(NCHUNK, SNB, K0F, 128).transpose(0, 3, 2, 1)
        ).reshape(NCHUNK * 128, K0F * SNB)
        # xrem[sc*128+32i+j, b] = xs[sc*1024+b, 768+j], replicated over i
        xrp = xs[:, K0F * 128 :].reshape(NCHUNK, SNB, K0R).transpose(0, 2, 1)
        xrem = np.zeros((NCHUNK, 128, SNB), dtype=fp8)
        for i in range(KH):
            xrem[:, 32 * i : 32 * i + K0R, :] = xrp
        in_maps.append(
            {"xmain": xmain, "xrem": xrem.reshape(NCHUNK * 128, SNB), **common}
        )
    return in_maps


def assemble_output(res):
    # out dram is the flat SBUF layout [128, 64, 10]; row rg*128+p of the
    # core's shard lives at out[p, rg, :].
    parts = []
    for i in range(N_CORES):
        o = np.asarray(res.results[i]["out"], dtype=np.float32)
        parts.append(o.transpose(1, 0, 2).reshape(BC, C))
    return np.concatenate(parts, axis=0)


def kernel(x, W1, b1, W2, b2, W3, b3, W4, b4):
    in_maps = make_in_maps(x, W1, b1, W2, b2, W3, b3, W4, b4)
    nc = _get_nc()
    res = run_bass_kernel_spmd(nc, in_maps, list(range(N_CORES)))
    return assemble_output(res)

